# revision 1
# baseline (speedup 1.0000x reference)
"""Multi-head self-attention (B=2, S=2048, D=1024, H=16, causal) on 8 trn2 cores.

Sharding: core c handles batch b = c//4 and 4 heads (c%4)*4 .. +4.
Per-core device program (all-transposed layout, no on-chip transposes):
  QT[dh,S] = Wq^T x^T, KT = Wk^T x^T   (heads stacked in pairs of 2 -> 128 parts)
  V[S,dvh] = (x^T)^T Wv, with a ones column appended (V\' is [128,65] per block)
  per head, per q-half, per key block kb:
    ST[kp, q] = K Q^T for q >= 128*kb   (causal, scores transposed)
    exp on ScalarE (no max subtraction -- scores are provably < ~3 here);
    the diagonal block is masked multiplicatively on the bf16 exp output
    outT[65, q] += V\'^T exp(ST)        (row 64 = softmax denominator)
Host: out = (outT[:64]/outT[64]).T + bv, reassembled into [B,S,H*dvh].
"""

import os
import sys

import numpy as np
import ml_dtypes

for _p in ("/opt/trn_rl_repo",):
    if _p not in sys.path and os.path.isdir(_p):
        sys.path.insert(0, _p)

B, S, D = 2, 2048, 1024
H = 16
DH = 64           # qk head dim
DVH = 64          # v head dim
HPC = 4           # heads per core
NCORES = 8
SCALE = 1.0 / 8.0  # 1/sqrt(dvh)
QH = 512           # q-quarter size

BF16 = ml_dtypes.bfloat16

_CACHE = {}


def _build_program(repeat=1):
    import concourse.tile as tile
    from concourse import bacc, mybir

    dt = mybir.dt
    nc = bacc.Bacc("TRN2", target_bir_lowering=False, debug=False,
                   num_devices=NCORES)

    xt_d = nc.dram_tensor("xt", [8, 128, S], dt.bfloat16, kind="ExternalInput").ap()
    wq_d = nc.dram_tensor("wq", [8, 128, 256], dt.bfloat16, kind="ExternalInput").ap()
    wk_d = nc.dram_tensor("wk", [8, 128, 256], dt.bfloat16, kind="ExternalInput").ap()
    wv_d = nc.dram_tensor("wv", [8, 128, 256], dt.bfloat16, kind="ExternalInput").ap()
    bq_d = nc.dram_tensor("bq", [128, 2], dt.float32, kind="ExternalInput").ap()
    bk_d = nc.dram_tensor("bk", [128, 2], dt.float32, kind="ExternalInput").ap()
    am_d = nc.dram_tensor("amask", [128, 128], dt.bfloat16, kind="ExternalInput").ap()
    out_d = nc.dram_tensor("out", [HPC, 65, S], dt.float32, kind="ExternalOutput").ap()

    for _ in range(repeat):
        _build_body(nc, tile, mybir,
                    xt_d, wq_d, wk_d, wv_d, bq_d, bk_d, am_d, out_d)

    nc.compile()
    return nc


def _build_body(nc, tile, mybir, xt_d, wq_d, wk_d, wv_d, bq_d, bk_d, am_d, out_d):
    dt = mybir.dt
    Exp = mybir.ActivationFunctionType.Exp

    with tile.TileContext(nc) as tc:
        with (
            tc.tile_pool(name="const", bufs=1) as const,
            tc.tile_pool(name="expp", bufs=8) as expp,
            tc.tile_pool(name="osb", bufs=2) as osb,
        ):
            xt_sb = const.tile([128, 8, S], dt.bfloat16)
            wq_sb = const.tile([128, 8, 256], dt.bfloat16)
            wk_sb = const.tile([128, 8, 256], dt.bfloat16)
            wv_sb = const.tile([128, 8, 256], dt.bfloat16)
            bq_sb = const.tile([128, 2], dt.float32)
            bk_sb = const.tile([128, 2], dt.float32)
            am_sb = const.tile([128, 128], dt.bfloat16)
            qt_sb = const.tile([128, 2, S], dt.bfloat16)
            kt_sb = const.tile([128, 2, S], dt.bfloat16)
            v_sb = const.tile([128, 16, HPC, 65], dt.bfloat16)

            # DMA issue order matters: HWDGE descriptor-gen serializes per
            # ring (~0.6us each), so issue first-needed tensors first and
            # alternate between the two HWDGE rings (sync + scalar).
            nc.sync.dma_start(wq_sb[:, :, :], wq_d.rearrange("c p n -> p c n"))
            nc.scalar.dma_start(wk_sb[:, :, :], wk_d.rearrange("c p n -> p c n"))
            rings = [nc.sync, nc.scalar]
            for ct in (0, 1):
                for kc in range(8):
                    rings[kc % 2].dma_start(
                        xt_sb[:, kc, 512 * ct:512 * (ct + 1)],
                        xt_d[kc, :, 512 * ct:512 * (ct + 1)])
            nc.sync.dma_start(bq_sb[:, :], bq_d)
            nc.scalar.dma_start(bk_sb[:, :], bk_d)
            nc.sync.dma_start(am_sb[:, :], am_d)
            nc.scalar.dma_start(wv_sb[:, :, :], wv_d.rearrange("c p n -> p c n"))
            for ct in (2, 3):
                for kc in range(8):
                    rings[kc % 2].dma_start(
                        xt_sb[:, kc, 512 * ct:512 * (ct + 1)],
                        xt_d[kc, :, 512 * ct:512 * (ct + 1)])

            def proj_qk_ct(pool, tag, p, ct, which):
                dst_sb, w_sb, b_sb = ((qt_sb, wq_sb, bq_sb),
                                      (kt_sb, wk_sb, bk_sb))[which]
                ps = pool.tile([128, 512], dt.float32, tag=tag, name="ps")
                for kc in range(8):
                    nc.tensor.matmul(
                        ps,
                        w_sb[:, kc, 128 * p:128 * (p + 1)],
                        xt_sb[:, kc, 512 * ct:512 * (ct + 1)],
                        start=(kc == 0), stop=(kc == 7),
                    )
                nc.vector.tensor_scalar_add(
                    dst_sb[:, p, 512 * ct:512 * (ct + 1)], ps, b_sb[:, p:p + 1])

            def proj_v(pool, tag, sc_lo, sc_hi):
                # V: [S, 4 heads x 64] natural layout + ones col appended
                for sc in range(sc_lo, sc_hi):
                    ps2 = pool.tile([128, HPC, 64], dt.float32, tag=tag, name="ps2")
                    for kc in range(8):
                        nc.tensor.matmul(
                            ps2,
                            xt_sb[:, kc, 128 * sc:128 * (sc + 1)],
                            wv_sb[:, kc, :],
                            start=(kc == 0), stop=(kc == 7),
                        )
                    nc.vector.tensor_copy(v_sb[:, sc, :, 0:64], ps2)

            nc.vector.memset(v_sb[:, :, :, 64], 1.0)
            # stp(2x2) + op(2) + pp(2x1) = 8 PSUM banks; projections own
            # their slots so filler never contends with the ST->exp chain
            stp = tc.alloc_tile_pool(name="stp", bufs=5, space="PSUM")
            op = tc.alloc_tile_pool(name="op", bufs=2, space="PSUM")
            pp = tc.alloc_tile_pool(name="pp", bufs=1, space="PSUM")

            # global software pipeline across all (head, q-half) units:
            # one rolling pending-AV queue so the exp->AV edge never drains
            from collections import deque
            pend = deque()
            cur_out = {}
            LAG = 3

            def emit_av_one():
                (u, h, h0, h1, kb, cq0, clen, isdiag, ext, is_last) = pend.popleft()
                if u not in cur_out:
                    cur_out[u] = op.tile([65, QH], dt.float32, tag="op",
                                         name="outp")
                outp = cur_out[u]
                segs = []
                s0 = cq0
                if isdiag:
                    segs.append((cq0, 128, True))
                    s0 = cq0 + 128
                while s0 < h1:
                    s1 = min((s0 // 512 + 1) * 512, h1)
                    segs.append((s0, s1 - s0, False))
                    s0 = s1
                for (g0, gl, isd) in segs:
                    nc.tensor.matmul(
                        outp[:, g0 - h0:g0 - h0 + gl],
                        v_sb[:, kb, h, :],
                        ext[:, g0 - cq0:g0 - cq0 + gl],
                        start=(kb == 0 and g0 % 512 == 0),
                        stop=(isd and kb % 4 == 3),
                    )
                if is_last:
                    h_, h0_ = h, h0
                    ot = osb.tile([65, QH], dt.float32, tag="ot", name="ot")
                    nc.vector.tensor_copy(ot, outp)
                    nc.sync.dma_start(out_d[h_, :, h0_:h0_ + QH], ot)
                    del cur_out[u]

            def attn_head(h, halves=(0, 1, 2, 3)):
                p, hi = h // 2, h % 2
                base = 64 * hi
                for qh in halves:
                    h0, h1 = QH * qh, QH * (qh + 1)
                    u = (h, qh)
                    kbs = [kb for kb in range(16) if max(128 * kb, h0) < h1]
                    for kb in kbs:
                        cq0 = max(128 * kb, h0)
                        clen = h1 - cq0
                        isdiag = 128 * kb >= h0
                        st = stp.tile([128, clen], dt.float32, tag="st", name="st")
                        n0 = 0
                        while n0 < clen:
                            nl = min(512, clen - n0)
                            nc.tensor.matmul(
                                st[:, n0:n0 + nl],
                                kt_sb[base:base + 64, p, 128 * kb:128 * kb + 128],
                                qt_sb[base:base + 64, p, cq0 + n0:cq0 + n0 + nl],
                                start=True, stop=True,
                            )
                            n0 += nl
                        ext = expp.tile([128, clen], dt.bfloat16, tag="ex", name="ext")
                        nc.scalar.activation(ext, st, Exp, scale=SCALE)
                        if isdiag:
                            nc.vector.tensor_mul(ext[:, 0:128], ext[:, 0:128],
                                                 am_sb)
                        pend.append((u, h, h0, h1, kb, cq0, clen, isdiag, ext,
                                     kb == kbs[-1]))
                        while len(pend) > LAG:
                            emit_av_one()

            def attn_drain():
                while pend:
                    emit_av_one()

            # priority layout: ramp projections, then attention interleaved
            # with just-in-time projections in their own PSUM pool
            for ct in (0, 1):
                proj_qk_ct(pp, "pp", 0, ct, 0)
                proj_qk_ct(pp, "pp", 0, ct, 1)
            proj_v(pp, "pp", 0, 8)
            attn_head(0, halves=(0, 1))
            proj_v(pp, "pp", 8, 16)
            for ct in (2, 3):
                proj_qk_ct(pp, "pp", 0, ct, 0)
                proj_qk_ct(pp, "pp", 0, ct, 1)
            attn_head(0, halves=(2, 3))
            attn_head(1)
            for ct in range(4):
                proj_qk_ct(pp, "pp", 1, ct, 0)
                proj_qk_ct(pp, "pp", 1, ct, 1)
            attn_head(2)
            attn_head(3)
            attn_drain()
            pp.release()
            op.release()
            stp.release()


def _get_program():
    if "nc" not in _CACHE:
        _CACHE["nc"] = _build_program()
    return _CACHE["nc"]


def make_in_maps(x, Wqk, bqk, Wv, bv):
    ii, jj = np.meshgrid(np.arange(128), np.arange(128), indexing="ij")
    amask = np.where(ii <= jj, 1.0, 0.0).astype(BF16)
    in_maps = []
    for c in range(NCORES):
        b, g = divmod(c, 4)
        cols = slice(256 * g, 256 * (g + 1))
        xt = np.ascontiguousarray(x[b].T).astype(BF16).reshape(8, 128, S)
        wq = np.ascontiguousarray(Wqk[:, cols]).astype(BF16).reshape(8, 128, 256)
        wk = np.ascontiguousarray(Wqk[:, D:][:, cols]).astype(BF16).reshape(8, 128, 256)
        wv = np.ascontiguousarray(Wv[:, cols]).astype(BF16).reshape(8, 128, 256)
        bq = np.ascontiguousarray(bqk[cols].reshape(2, 128).T).astype(np.float32)
        bk = np.ascontiguousarray(bqk[D:][cols].reshape(2, 128).T).astype(np.float32)
        in_maps.append({"xt": xt, "wq": wq, "wk": wk, "wv": wv,
                        "bq": bq, "bk": bk, "amask": amask})
    return in_maps


def assemble(per_core_out, bv):
    out = np.empty((B, S, H * DVH), np.float32)
    for c in range(NCORES):
        b, g = divmod(c, 4)
        o = per_core_out[c]  # [HPC, 65, S]
        for hh in range(HPC):
            hg = HPC * g + hh
            a = o[hh, :64, :] / o[hh, 64:65, :]
            out[b, :, DVH * hg:DVH * (hg + 1)] = a.T + bv[DVH * hg:DVH * (hg + 1)]
    return out


def kernel(x, Wqk, bqk, Wv, bv):
    from concourse.bass_utils import run_bass_kernel_spmd

    nc = _get_program()
    in_maps = make_in_maps(np.asarray(x, np.float32), np.asarray(Wqk, np.float32),
                           np.asarray(bqk, np.float32), np.asarray(Wv, np.float32),
                           np.asarray(bv, np.float32))
    trace = os.environ.get("MHA_TRACE", "0") == "1"
    try:
        res = run_bass_kernel_spmd(nc, in_maps, list(range(NCORES)), trace=trace)
    except Exception:
        if not trace:
            raise
        # trace path unavailable (e.g. no NTFF hook on this axon client)
        res = run_bass_kernel_spmd(nc, in_maps, list(range(NCORES)), trace=False)
    _CACHE["last_result"] = res
    return assemble([r["out"] for r in res.results], np.asarray(bv, np.float32))



# revision 2
# speedup vs baseline: 1.1496x; 1.1496x over previous
"""Multi-head self-attention (B=2, S=2048, D=1024, H=16, causal) on 8 trn2 cores.

Sharding: core c handles batch b = c//4 and 4 heads (c%4)*4 .. +4.
Per-core device program (all-transposed layout, no on-chip transposes):
  QT[dh,S] = Wq^T x^T, KT = Wk^T x^T   (heads stacked in pairs of 2 -> 128 parts)
  V[S,dvh] = (x^T)^T Wv, with a ones column appended (V' is [128,65] per block)
  per head, per q-quarter, per key block kb:
    ST[kp, q] = K Q^T for q >= 128*kb   (causal, scores transposed)
    exp: split across ScalarE (true exp) and VectorE (1-op Schraudolph
    bit-trick: u16 = round(A2*s + B2) whose bytes ARE bf16 exp(s*SCALE));
    the diagonal block is masked multiplicatively (GpSimd) on the bf16 output
    outT[65, q] += V'^T exp(ST)        (row 64 = softmax denominator)
  The two heads of a pair use PE row groups 0:64 / 64:128, so their score
  matmuls are issued adjacently to stream concurrently (row tiling).
Host: out = (outT[:64]/outT[64]).T + bv, reassembled into [B,S,H*dvh].
"""

import os
import sys

import numpy as np
import ml_dtypes

for _p in ("/opt/trn_rl_repo",):
    if _p not in sys.path and os.path.isdir(_p):
        sys.path.insert(0, _p)

B, S, D = 2, 2048, 1024
H = 16
DH = 64           # qk head dim
DVH = 64          # v head dim
HPC = 4           # heads per core
NCORES = 8
SCALE = 1.0 / 8.0  # 1/sqrt(dvh)
QH = 512           # q-quarter size

# Schraudolph-to-bf16: u16 z = A2*s_raw + B2; bf16 bits z<<16 ~= exp(SCALE*s).
# A2 = SCALE * log2(e) * 2^7; B2 = 127*2^7 - c16, c16 calibrated for min RMS
# (1.78% rms, 4.2% max rel err on scores in [-6*8, 3.5*8]).
EXP_A2 = SCALE * 1.4426950408889634 * 128.0
EXP_B2 = 16256.0 - 7.5

# Engine balance knobs (lane-element costs: ACT 0.833ns, DVE 1.042ns).
ACT_COEF = float(os.environ.get("MHA_ACT_COEF", 0.833))
DVE_COEF = float(os.environ.get("MHA_DVE_COEF", 1.042))
ACT_INIT = float(os.environ.get("MHA_ACT_INIT", 12288))  # V+out moves on ACT
DVE_INIT = float(os.environ.get("MHA_DVE_INIT", 8192))   # qt/kt moves on DVE
SKIP_EXP = os.environ.get("MHA_SKIP_EXP", "0") == "1"
MASK_ENGINE = os.environ.get("MHA_MASK_ENGINE", "gpsimd")
LAG = int(os.environ.get("MHA_LAG", "6"))

BF16 = ml_dtypes.bfloat16

_CACHE = {}


def _build_program(repeat=1):
    import concourse.tile as tile
    from concourse import bacc, mybir

    dt = mybir.dt
    nc = bacc.Bacc("TRN2", target_bir_lowering=False, debug=False,
                   num_devices=NCORES)

    xt_d = nc.dram_tensor("xt", [8, 128, S], dt.bfloat16, kind="ExternalInput").ap()
    wq_d = nc.dram_tensor("wq", [8, 128, 256], dt.bfloat16, kind="ExternalInput").ap()
    wk_d = nc.dram_tensor("wk", [8, 128, 256], dt.bfloat16, kind="ExternalInput").ap()
    wv_d = nc.dram_tensor("wv", [8, 128, 256], dt.bfloat16, kind="ExternalInput").ap()
    bq_d = nc.dram_tensor("bq", [128, 2], dt.float32, kind="ExternalInput").ap()
    bk_d = nc.dram_tensor("bk", [128, 2], dt.float32, kind="ExternalInput").ap()
    am_d = nc.dram_tensor("amask", [128, 128], dt.bfloat16, kind="ExternalInput").ap()
    out_d = nc.dram_tensor("out", [HPC, 65, S], dt.float32, kind="ExternalOutput").ap()

    for _ in range(repeat):
        _build_body(nc, tile, mybir,
                    xt_d, wq_d, wk_d, wv_d, bq_d, bk_d, am_d, out_d)

    nc.compile()
    return nc


def _build_body(nc, tile, mybir, xt_d, wq_d, wk_d, wv_d, bq_d, bk_d, am_d, out_d):
    dt = mybir.dt
    Exp = mybir.ActivationFunctionType.Exp
    Copy = mybir.ActivationFunctionType.Copy
    Alu = mybir.AluOpType

    with tile.TileContext(nc) as tc:
        with (
            tc.tile_pool(name="const", bufs=1) as const,
            tc.tile_pool(name="expp", bufs=10) as expp,
            tc.tile_pool(name="osb", bufs=2) as osb,
        ):
            xt_sb = const.tile([128, 8, S], dt.bfloat16)
            wq_sb = const.tile([128, 8, 256], dt.bfloat16)
            wk_sb = const.tile([128, 8, 256], dt.bfloat16)
            wv_sb = const.tile([128, 8, 256], dt.bfloat16)
            bq_sb = const.tile([128, 2], dt.float32)
            bk_sb = const.tile([128, 2], dt.float32)
            am_sb = const.tile([128, 128], dt.bfloat16)
            qt_sb = const.tile([128, 2, S], dt.bfloat16)
            kt_sb = const.tile([128, 2, S], dt.bfloat16)
            v_sb = const.tile([128, 16, HPC, 65], dt.bfloat16)

            # DMA issue order matters: HWDGE descriptor-gen serializes per
            # ring (~0.6us each), so issue first-needed tensors first and
            # alternate between the two HWDGE rings (sync + scalar).
            nc.sync.dma_start(wq_sb[:, :, :], wq_d.rearrange("c p n -> p c n"))
            nc.scalar.dma_start(wk_sb[:, :, :], wk_d.rearrange("c p n -> p c n"))
            rings = [nc.sync, nc.scalar]
            for ct in (0, 1):
                for kc in range(8):
                    rings[kc % 2].dma_start(
                        xt_sb[:, kc, 512 * ct:512 * (ct + 1)],
                        xt_d[kc, :, 512 * ct:512 * (ct + 1)])
            nc.sync.dma_start(bq_sb[:, :], bq_d)
            nc.scalar.dma_start(bk_sb[:, :], bk_d)
            nc.sync.dma_start(am_sb[:, :], am_d)
            nc.scalar.dma_start(wv_sb[:, :, :], wv_d.rearrange("c p n -> p c n"))
            for ct in (2, 3):
                for kc in range(8):
                    rings[kc % 2].dma_start(
                        xt_sb[:, kc, 512 * ct:512 * (ct + 1)],
                        xt_d[kc, :, 512 * ct:512 * (ct + 1)])

            def proj_qk_ct(pool, tag, p, ct, which):
                dst_sb, w_sb, b_sb = ((qt_sb, wq_sb, bq_sb),
                                      (kt_sb, wk_sb, bk_sb))[which]
                ps = pool.tile([128, 512], dt.float32, tag=tag, name="ps")
                for kc in range(8):
                    nc.tensor.matmul(
                        ps,
                        w_sb[:, kc, 128 * p:128 * (p + 1)],
                        xt_sb[:, kc, 512 * ct:512 * (ct + 1)],
                        start=(kc == 0), stop=(kc == 7),
                    )
                nc.vector.tensor_scalar_add(
                    dst_sb[:, p, 512 * ct:512 * (ct + 1)], ps, b_sb[:, p:p + 1])

            def proj_v(pool, tag, sc_lo, sc_hi):
                # V: [S, 4 heads x 64] natural layout + ones col appended
                for sc in range(sc_lo, sc_hi):
                    ps2 = pool.tile([128, HPC, 64], dt.float32, tag=tag, name="ps2")
                    for kc in range(8):
                        nc.tensor.matmul(
                            ps2,
                            xt_sb[:, kc, 128 * sc:128 * (sc + 1)],
                            wv_sb[:, kc, :],
                            start=(kc == 0), stop=(kc == 7),
                        )
                    nc.scalar.activation(v_sb[:, sc, :, 0:64], ps2, Copy)

            nc.vector.memset(v_sb[:, :, :, 64], 1.0)
            # stp(4) + op(3) + pp(1) = 8 PSUM banks; projections own their
            # slot so filler never contends with the ST->exp chain
            stp = tc.alloc_tile_pool(name="stp", bufs=4, space="PSUM")
            op = tc.alloc_tile_pool(name="op", bufs=3, space="PSUM")
            pp = tc.alloc_tile_pool(name="pp", bufs=1, space="PSUM")

            # global software pipeline across all (head, q-quarter) units:
            # one rolling pending-AV queue so the exp->AV edge never drains
            from collections import deque
            pend = deque()
            cur_out = {}
            # exp engine balance: greedy weighted assignment per unit
            ebal = {"act": ACT_INIT * ACT_COEF, "dve": DVE_INIT * DVE_COEF}

            def emit_exp(st, clen, isdiag):
                """exp(SCALE*st) -> bf16 tile; returns the AP to feed AV."""
                if SKIP_EXP:
                    ext = expp.tile([128, clen], dt.bfloat16, tag="ex", name="ext")
                    return ext
                ca = ebal["act"] + clen * ACT_COEF
                cd = ebal["dve"] + clen * DVE_COEF
                if ca <= cd:
                    ebal["act"] = ca
                    ext = expp.tile([128, clen], dt.bfloat16, tag="ex", name="ext")
                    nc.scalar.activation(ext, st, Exp, scale=SCALE)
                else:
                    ebal["dve"] = cd
                    extu = expp.tile([128, clen], dt.uint16, tag="ex", name="extu")
                    nc.vector.tensor_scalar(extu, st, EXP_A2, EXP_B2,
                                            Alu.mult, Alu.add)
                    ext = extu.bitcast(dt.bfloat16)
                if isdiag:
                    if MASK_ENGINE == "gpsimd":
                        nc.gpsimd.tensor_mul(ext[:, 0:128], ext[:, 0:128], am_sb)
                    else:
                        nc.vector.tensor_mul(ext[:, 0:128], ext[:, 0:128], am_sb)
                return ext

            def emit_av_one():
                (u, h, h0, h1, kb, cq0, clen, isdiag, ext, is_last) = pend.popleft()
                if u not in cur_out:
                    cur_out[u] = op.tile([65, QH], dt.float32, tag="op",
                                         name="outp")
                outp = cur_out[u]
                segs = []
                s0 = cq0
                if isdiag:
                    segs.append((cq0, 128, True))
                    s0 = cq0 + 128
                while s0 < h1:
                    s1 = min((s0 // 512 + 1) * 512, h1)
                    segs.append((s0, s1 - s0, False))
                    s0 = s1
                for (g0, gl, isd) in segs:
                    nc.tensor.matmul(
                        outp[:, g0 - h0:g0 - h0 + gl],
                        v_sb[:, kb, h, :],
                        ext[:, g0 - cq0:g0 - cq0 + gl],
                        start=(kb == 0 and g0 % 512 == 0),
                        stop=(isd and kb % 4 == 3),
                    )
                if is_last:
                    h_, h0_ = h, h0
                    ot = osb.tile([65, QH], dt.float32, tag="ot", name="ot")
                    nc.scalar.activation(ot, outp, Copy)
                    nc.sync.dma_start(out_d[h_, :, h0_:h0_ + QH], ot)
                    del cur_out[u]

            def attn_pair(p, halves=(0, 1, 2, 3)):
                # heads 2p (PE rows 0:64) and 2p+1 (rows 64:128): issue their
                # score matmuls adjacently so the row tiles stream concurrently
                for qh in halves:
                    h0, h1 = QH * qh, QH * (qh + 1)
                    kbs = [kb for kb in range(16) if max(128 * kb, h0) < h1]
                    for kb in kbs:
                        cq0 = max(128 * kb, h0)
                        clen = h1 - cq0
                        isdiag = 128 * kb >= h0
                        sts = []
                        for hi in (0, 1):
                            base = 64 * hi
                            st = stp.tile([128, clen], dt.float32, tag="st",
                                          name="st")
                            n0 = 0
                            while n0 < clen:
                                nl = min(512, clen - n0)
                                nc.tensor.matmul(
                                    st[:, n0:n0 + nl],
                                    kt_sb[base:base + 64, p,
                                          128 * kb:128 * kb + 128],
                                    qt_sb[base:base + 64, p,
                                          cq0 + n0:cq0 + n0 + nl],
                                    start=True, stop=True,
                                )
                                n0 += nl
                            sts.append(st)
                        for hi in (0, 1):
                            h = 2 * p + hi
                            ext = emit_exp(sts[hi], clen, isdiag)
                            pend.append(((h, qh), h, h0, h1, kb, cq0, clen,
                                         isdiag, ext, kb == kbs[-1]))
                        while len(pend) > LAG:
                            emit_av_one()

            def attn_drain():
                while pend:
                    emit_av_one()

            # priority layout: ramp projections, then attention interleaved
            # with just-in-time projections in their own PSUM pool
            for ct in (0, 1):
                proj_qk_ct(pp, "pp", 0, ct, 0)
                proj_qk_ct(pp, "pp", 0, ct, 1)
            proj_v(pp, "pp", 0, 8)
            attn_pair(0, halves=(0, 1))
            proj_v(pp, "pp", 8, 16)
            for ct in (2, 3):
                proj_qk_ct(pp, "pp", 0, ct, 0)
                proj_qk_ct(pp, "pp", 0, ct, 1)
            attn_pair(0, halves=(2, 3))
            for ct in range(4):
                proj_qk_ct(pp, "pp", 1, ct, 0)
                proj_qk_ct(pp, "pp", 1, ct, 1)
            attn_pair(1)
            attn_drain()
            pp.release()
            op.release()
            stp.release()


def _get_program():
    if "nc" not in _CACHE:
        _CACHE["nc"] = _build_program()
    return _CACHE["nc"]


def make_in_maps(x, Wqk, bqk, Wv, bv):
    ii, jj = np.meshgrid(np.arange(128), np.arange(128), indexing="ij")
    amask = np.where(ii <= jj, 1.0, 0.0).astype(BF16)
    in_maps = []
    for c in range(NCORES):
        b, g = divmod(c, 4)
        cols = slice(256 * g, 256 * (g + 1))
        xt = np.ascontiguousarray(x[b].T).astype(BF16).reshape(8, 128, S)
        wq = np.ascontiguousarray(Wqk[:, cols]).astype(BF16).reshape(8, 128, 256)
        wk = np.ascontiguousarray(Wqk[:, D:][:, cols]).astype(BF16).reshape(8, 128, 256)
        wv = np.ascontiguousarray(Wv[:, cols]).astype(BF16).reshape(8, 128, 256)
        bq = np.ascontiguousarray(bqk[cols].reshape(2, 128).T).astype(np.float32)
        bk = np.ascontiguousarray(bqk[D:][cols].reshape(2, 128).T).astype(np.float32)
        in_maps.append({"xt": xt, "wq": wq, "wk": wk, "wv": wv,
                        "bq": bq, "bk": bk, "amask": amask})
    return in_maps


def assemble(per_core_out, bv):
    out = np.empty((B, S, H * DVH), np.float32)
    for c in range(NCORES):
        b, g = divmod(c, 4)
        o = per_core_out[c]  # [HPC, 65, S]
        for hh in range(HPC):
            hg = HPC * g + hh
            a = o[hh, :64, :] / o[hh, 64:65, :]
            out[b, :, DVH * hg:DVH * (hg + 1)] = a.T + bv[DVH * hg:DVH * (hg + 1)]
    return out


def kernel(x, Wqk, bqk, Wv, bv):
    from concourse.bass_utils import run_bass_kernel_spmd

    nc = _get_program()
    in_maps = make_in_maps(np.asarray(x, np.float32), np.asarray(Wqk, np.float32),
                           np.asarray(bqk, np.float32), np.asarray(Wv, np.float32),
                           np.asarray(bv, np.float32))
    trace = os.environ.get("MHA_TRACE", "0") == "1"
    try:
        res = run_bass_kernel_spmd(nc, in_maps, list(range(NCORES)), trace=trace)
    except Exception:
        if not trace:
            raise
        # trace path unavailable (e.g. no NTFF hook on this axon client)
        res = run_bass_kernel_spmd(nc, in_maps, list(range(NCORES)), trace=False)
    _CACHE["last_result"] = res
    return assemble([r["out"] for r in res.results], np.asarray(bv, np.float32))


# revision 4
# speedup vs baseline: 1.6593x; 1.4434x over previous
"""Multi-head self-attention (B=2, S=2048, D=1024, H=16, causal) on 8 trn2 cores.

Sharding: core c handles batch b = c//4 and 4 heads (c%4)*4 .. +4.
Per-core device program (all-transposed layout, no on-chip transposes):
  QT[dh,S] = Wq^T x^T, KT = Wk^T x^T   (heads stacked in pairs of 2 -> 128 parts)
  V[S,dvh] = (x^T)^T Wv, with a ones column appended (V' is [128,65] per block)
  per head-pair, per q-quarter, per key block kb:
    ST[kp, q] for both heads into one 2-bank psum tile [128, 2, 512]
    (heads use PE row groups 0:64 / 64:128; adjacent issue -> concurrent
    row-tile streaming)
    exp: split across ScalarE (true exp) and VectorE (1-op Schraudolph
    bit-trick: u16 = round(A2*s + B2) whose bytes ARE bf16 exp(s*SCALE));
    clen=512 units exp both heads in ONE instruction;
    the diagonal block is masked multiplicatively (GpSimd) on the bf16 output
    outT[65, q] += V'^T exp(ST)        (row 64 = softmax denominator)
Host: out = (outT[:64]/outT[64]).T + bv, reassembled into [B,S,H*dvh].
"""

import os
import sys

import numpy as np
import ml_dtypes

for _p in ("/opt/trn_rl_repo",):
    if _p not in sys.path and os.path.isdir(_p):
        sys.path.insert(0, _p)

B, S, D = 2, 2048, 1024
H = 16
DH = 64           # qk head dim
DVH = 64          # v head dim
HPC = 4           # heads per core
NCORES = 8
SCALE = 1.0 / 8.0  # 1/sqrt(dvh)
QH = 512           # q-quarter size

# Schraudolph-to-bf16: u16 z = A2*s_raw + B2; bf16 bits z<<16 ~= exp(SCALE*s).
# A2 = SCALE * log2(e) * 2^7; B2 = 127*2^7 - c16, c16 calibrated for min RMS
# (1.78% rms, 4.2% max rel err on scores in [-48, 28]).
EXP_A2 = SCALE * 1.4426950408889634 * 128.0
EXP_B2 = 16256.0 - 7.5

# Engine balance knobs (lane-element costs: ACT 0.833ns, DVE 1.042ns).
ACT_COEF = float(os.environ.get("MHA_ACT_COEF", 0.833))
DVE_COEF = float(os.environ.get("MHA_DVE_COEF", 1.042))
ACT_INIT = float(os.environ.get("MHA_ACT_INIT", 12288))  # V+out moves on ACT
DVE_INIT = float(os.environ.get("MHA_DVE_INIT", 8192))   # qt/kt moves on DVE
SKIP_EXP = os.environ.get("MHA_SKIP_EXP", "0") == "1"
MASK_ENGINE = os.environ.get("MHA_MASK_ENGINE", "gpsimd")
LAG = int(os.environ.get("MHA_LAG", "6"))

BF16 = ml_dtypes.bfloat16

_CACHE = {}


def _build_program(repeat=1):
    import concourse.tile as tile
    from concourse import bacc, mybir

    dt = mybir.dt
    nc = bacc.Bacc("TRN2", target_bir_lowering=False, debug=False,
                   num_devices=NCORES)

    xt_d = nc.dram_tensor("xt", [8, 128, S], dt.bfloat16, kind="ExternalInput").ap()
    wq_d = nc.dram_tensor("wq", [128, 8, 256], dt.bfloat16, kind="ExternalInput").ap()
    wk_d = nc.dram_tensor("wk", [128, 8, 256], dt.bfloat16, kind="ExternalInput").ap()
    wv_d = nc.dram_tensor("wv", [128, 8, 256], dt.bfloat16, kind="ExternalInput").ap()
    bq_d = nc.dram_tensor("bq", [128, 2], dt.float32, kind="ExternalInput").ap()
    bk_d = nc.dram_tensor("bk", [128, 2], dt.float32, kind="ExternalInput").ap()
    am_d = nc.dram_tensor("amask", [128, 128], dt.bfloat16, kind="ExternalInput").ap()
    out_d = nc.dram_tensor("out", [HPC, 65, S], dt.float32, kind="ExternalOutput").ap()

    for _ in range(repeat):
        _build_body(nc, tile, mybir,
                    xt_d, wq_d, wk_d, wv_d, bq_d, bk_d, am_d, out_d)

    nc.compile()
    return nc


def _build_body(nc, tile, mybir, xt_d, wq_d, wk_d, wv_d, bq_d, bk_d, am_d, out_d):
    dt = mybir.dt
    Exp = mybir.ActivationFunctionType.Exp
    Copy = mybir.ActivationFunctionType.Copy
    Alu = mybir.AluOpType

    with tile.TileContext(nc) as tc:
        with (
            tc.tile_pool(name="const", bufs=1) as const,
            tc.tile_pool(name="expp", bufs=8) as expp,
            tc.tile_pool(name="osb", bufs=2) as osb,
        ):
            xt_sb = const.tile([128, 8, S], dt.bfloat16)
            wq_sb = const.tile([128, 8, 256], dt.bfloat16)
            wk_sb = const.tile([128, 8, 256], dt.bfloat16)
            wv_sb = const.tile([128, 8, 256], dt.bfloat16)
            bq_sb = const.tile([128, 2], dt.float32)
            bk_sb = const.tile([128, 2], dt.float32)
            am_sb = const.tile([128, 128], dt.bfloat16)
            qt_sb = const.tile([128, 2, S], dt.bfloat16)
            kt_sb = const.tile([128, 2, S], dt.bfloat16)
            v_sb = const.tile([128, 16, HPC, 65], dt.bfloat16)

            # DMA issue order matters: first-needed tensors first, alternating
            # between the two HWDGE rings (sync + scalar). Weights are stored
            # partition-major in DRAM so every transfer has 4KB/partition
            # contiguous lines; xt is loaded in 1024-col halves (2KB lines).
            nc.sync.dma_start(wq_sb[:, :, :], wq_d)
            nc.scalar.dma_start(wk_sb[:, :, :], wk_d)
            rings = [nc.sync, nc.scalar]
            for kc in range(8):
                rings[kc % 2].dma_start(xt_sb[:, kc, 0:1024],
                                        xt_d[kc, :, 0:1024])
            nc.sync.dma_start(bq_sb[:, :], bq_d)
            nc.scalar.dma_start(bk_sb[:, :], bk_d)
            nc.sync.dma_start(am_sb[:, :], am_d)
            nc.scalar.dma_start(wv_sb[:, :, :], wv_d)
            for kc in range(8):
                rings[kc % 2].dma_start(xt_sb[:, kc, 1024:2048],
                                        xt_d[kc, :, 1024:2048])

            def proj_qk_ct(pool, tag, p, ct, which):
                dst_sb, w_sb, b_sb = ((qt_sb, wq_sb, bq_sb),
                                      (kt_sb, wk_sb, bk_sb))[which]
                ps = pool.tile([128, 512], dt.float32, tag=tag, name="ps")
                for kc in range(8):
                    nc.tensor.matmul(
                        ps,
                        w_sb[:, kc, 128 * p:128 * (p + 1)],
                        xt_sb[:, kc, 512 * ct:512 * (ct + 1)],
                        start=(kc == 0), stop=(kc == 7),
                    )
                nc.vector.tensor_scalar_add(
                    dst_sb[:, p, 512 * ct:512 * (ct + 1)], ps, b_sb[:, p:p + 1])

            def proj_v(pool, tag, sc_lo, sc_hi):
                # V: [S, 4 heads x 64] natural layout + ones col appended
                for sc in range(sc_lo, sc_hi):
                    ps2 = pool.tile([128, HPC, 64], dt.float32, tag=tag, name="ps2")
                    for kc in range(8):
                        nc.tensor.matmul(
                            ps2,
                            xt_sb[:, kc, 128 * sc:128 * (sc + 1)],
                            wv_sb[:, kc, :],
                            start=(kc == 0), stop=(kc == 7),
                        )
                    nc.scalar.activation(v_sb[:, sc, :, 0:64], ps2, Copy)

            nc.vector.memset(v_sb[:, :, :, 64], 1.0)
            # stp(2x2banks) + op(3) + pp(1) = 8 PSUM banks; projections own
            # their slot so filler never contends with the ST->exp chain
            stp = tc.alloc_tile_pool(name="stp", bufs=2, space="PSUM")
            op = tc.alloc_tile_pool(name="op", bufs=3, space="PSUM")
            pp = tc.alloc_tile_pool(name="pp", bufs=1, space="PSUM")

            # global software pipeline across all (head, q-quarter) units:
            # one rolling pending-AV queue so the exp->AV edge never drains
            from collections import deque
            pend = deque()
            cur_out = {}
            # exp engine balance: greedy weighted assignment per unit
            ebal = {"act": ACT_INIT * ACT_COEF, "dve": DVE_INIT * DVE_COEF}

            def pick_engine(nels):
                ca = ebal["act"] + nels * ACT_COEF
                cd = ebal["dve"] + nels * DVE_COEF
                if ca <= cd:
                    ebal["act"] = ca
                    return "act"
                ebal["dve"] = cd
                return "dve"

            def emit_exp_pair(st2, clen, isdiag):
                """exp both heads' scores; returns (ext_even, ext_odd) APs."""
                ext2 = expp.tile([128, 2, 512], dt.uint16, tag="ex", name="ext2")
                extb = ext2.bitcast(dt.bfloat16)
                if not SKIP_EXP:
                    if clen == 512:
                        eng = pick_engine(1024)
                        if eng == "act":
                            nc.scalar.activation(extb, st2, Exp, scale=SCALE)
                        else:
                            nc.vector.tensor_scalar(ext2, st2, EXP_A2, EXP_B2,
                                                    Alu.mult, Alu.add)
                    else:
                        for hi in (0, 1):
                            eng = pick_engine(clen)
                            if eng == "act":
                                nc.scalar.activation(
                                    extb[:, hi, 0:clen], st2[:, hi, 0:clen],
                                    Exp, scale=SCALE)
                            else:
                                nc.vector.tensor_scalar(
                                    ext2[:, hi, 0:clen], st2[:, hi, 0:clen],
                                    EXP_A2, EXP_B2, Alu.mult, Alu.add)
                    if isdiag:
                        meng = nc.gpsimd if MASK_ENGINE == "gpsimd" else nc.vector
                        for hi in (0, 1):
                            meng.tensor_mul(extb[:, hi, 0:128],
                                            extb[:, hi, 0:128], am_sb)
                return extb[:, 0, :], extb[:, 1, :]

            def emit_av_one():
                (u, h, h0, h1, kb, cq0, clen, isdiag, ext, is_last) = pend.popleft()
                if u not in cur_out:
                    cur_out[u] = op.tile([65, QH], dt.float32, tag="op",
                                         name="outp")
                outp = cur_out[u]
                segs = []
                s0 = cq0
                if isdiag:
                    segs.append((cq0, 128, True))
                    s0 = cq0 + 128
                while s0 < h1:
                    s1 = min((s0 // 512 + 1) * 512, h1)
                    segs.append((s0, s1 - s0, False))
                    s0 = s1
                for (g0, gl, isd) in segs:
                    nc.tensor.matmul(
                        outp[:, g0 - h0:g0 - h0 + gl],
                        v_sb[:, kb, h, :],
                        ext[:, g0 - cq0:g0 - cq0 + gl],
                        start=(kb == 0 and g0 % 512 == 0),
                        stop=(isd and kb % 4 == 3),
                    )
                if is_last:
                    h_, h0_ = h, h0
                    ot = osb.tile([65, QH], dt.float32, tag="ot", name="ot")
                    nc.scalar.activation(ot, outp, Copy)
                    nc.sync.dma_start(out_d[h_, :, h0_:h0_ + QH], ot)
                    del cur_out[u]

            def attn_pair(p, halves=(0, 1, 2, 3)):
                # heads 2p (PE rows 0:64) and 2p+1 (rows 64:128): issue their
                # score matmuls adjacently so the row tiles stream concurrently
                for qh in halves:
                    h0, h1 = QH * qh, QH * (qh + 1)
                    kbs = [kb for kb in range(16) if max(128 * kb, h0) < h1]
                    for kb in kbs:
                        cq0 = max(128 * kb, h0)
                        clen = h1 - cq0
                        isdiag = 128 * kb >= h0
                        st2 = stp.tile([128, 2, 512], dt.float32, tag="st",
                                       name="st2")
                        for hi in (0, 1):
                            base = 64 * hi
                            nc.tensor.matmul(
                                st2[:, hi, 0:clen],
                                kt_sb[base:base + 64, p,
                                      128 * kb:128 * kb + 128],
                                qt_sb[base:base + 64, p, cq0:cq0 + clen],
                                start=True, stop=True,
                            )
                        exts = emit_exp_pair(st2, clen, isdiag)
                        for hi in (0, 1):
                            h = 2 * p + hi
                            # ext AP sliced to the valid clen columns
                            pend.append(((h, qh), h, h0, h1, kb, cq0, clen,
                                         isdiag, exts[hi][:, 0:clen],
                                         kb == kbs[-1]))
                        while len(pend) > LAG:
                            emit_av_one()

            def attn_drain():
                while pend:
                    emit_av_one()

            # priority layout: ramp projections, then attention interleaved
            # with just-in-time projections in their own PSUM pool
            for ct in (0, 1):
                proj_qk_ct(pp, "pp", 0, ct, 0)
                proj_qk_ct(pp, "pp", 0, ct, 1)
            proj_v(pp, "pp", 0, 8)
            attn_pair(0, halves=(0, 1))
            proj_v(pp, "pp", 8, 16)
            for ct in (2, 3):
                proj_qk_ct(pp, "pp", 0, ct, 0)
                proj_qk_ct(pp, "pp", 0, ct, 1)
            attn_pair(0, halves=(2, 3))
            for ct in range(4):
                proj_qk_ct(pp, "pp", 1, ct, 0)
                proj_qk_ct(pp, "pp", 1, ct, 1)
            attn_pair(1)
            attn_drain()
            pp.release()
            op.release()
            stp.release()


def _get_program():
    if "nc" not in _CACHE:
        _CACHE["nc"] = _build_program()
    return _CACHE["nc"]


def make_in_maps(x, Wqk, bqk, Wv, bv):
    ii, jj = np.meshgrid(np.arange(128), np.arange(128), indexing="ij")
    amask = np.where(ii <= jj, 1.0, 0.0).astype(BF16)

    def wlayout(w):  # [1024, 256] -> [128 parts, 8 chunks, 256] contiguous
        return np.ascontiguousarray(
            w.reshape(8, 128, 256).transpose(1, 0, 2)).astype(BF16)

    in_maps = []
    for c in range(NCORES):
        b, g = divmod(c, 4)
        cols = slice(256 * g, 256 * (g + 1))
        xt = np.ascontiguousarray(x[b].T).astype(BF16).reshape(8, 128, S)
        wq = wlayout(Wqk[:, cols])
        wk = wlayout(Wqk[:, D:][:, cols])
        wv = wlayout(Wv[:, cols])
        bq = np.ascontiguousarray(bqk[cols].reshape(2, 128).T).astype(np.float32)
        bk = np.ascontiguousarray(bqk[D:][cols].reshape(2, 128).T).astype(np.float32)
        in_maps.append({"xt": xt, "wq": wq, "wk": wk, "wv": wv,
                        "bq": bq, "bk": bk, "amask": amask})
    return in_maps


def assemble(per_core_out, bv):
    out = np.empty((B, S, H * DVH), np.float32)
    for c in range(NCORES):
        b, g = divmod(c, 4)
        o = per_core_out[c]  # [HPC, 65, S]
        for hh in range(HPC):
            hg = HPC * g + hh
            a = o[hh, :64, :] / o[hh, 64:65, :]
            out[b, :, DVH * hg:DVH * (hg + 1)] = a.T + bv[DVH * hg:DVH * (hg + 1)]
    return out


def kernel(x, Wqk, bqk, Wv, bv):
    from concourse.bass_utils import run_bass_kernel_spmd

    nc = _get_program()
    in_maps = make_in_maps(np.asarray(x, np.float32), np.asarray(Wqk, np.float32),
                           np.asarray(bqk, np.float32), np.asarray(Wv, np.float32),
                           np.asarray(bv, np.float32))
    trace = os.environ.get("MHA_TRACE", "0") == "1"
    try:
        res = run_bass_kernel_spmd(nc, in_maps, list(range(NCORES)), trace=trace)
    except Exception:
        if not trace:
            raise
        # trace path unavailable (e.g. no NTFF hook on this axon client)
        res = run_bass_kernel_spmd(nc, in_maps, list(range(NCORES)), trace=False)
    _CACHE["last_result"] = res
    return assemble([r["out"] for r in res.results], np.asarray(bv, np.float32))
